# revision 28
# baseline (speedup 1.0000x reference)
"""Trainium2 Bass kernel for nn_ReallocationMapEncoder.

The reference network is three NAC layers (y = x @ (tanh(W_hat)*sigmoid(M_hat)).T)
applied to a [nsteps, nsyms, nsyms, 3] grid of normalized (t, a, b) indices,
plus a gb broadcast on the trailing axis. NAC is linear in x, so the whole
network collapses to one effective matrix Weff = W3 @ W2 @ W1 of shape [2, 3]:

    y[t, a, b, c] = gb[c] + (t/2)*Weff[c,0] + (a/2048)*Weff[c,1] + (b/2048)*Weff[c,2]

The output [2, 2048, 2048, 2] f32 (67 MB) is a separable affine ramp; the kernel
is purely output-write-bandwidth bound (memory regime).

Device strategy (8 cores, data-parallel on the `a` axis, 256 rows each, so each
core writes 8.4 MB): a DVE iota builds J[p, j] = j; every output chunk
[128 a-rows, fsz free elems at c-stride] is a fused DVE tensor_scalar

    out[p, b, c] = J[p, b] * (Weff[c,2]/nsyms) + bias[p, (t,blk,c)]

where bias (a tiny [128, 8] per-core input) folds the gb/t/a terms.

Perf structure (16.4 us on the CoreSim cost model vs 32.6 us for the
all-SWDGE baseline; the ~358 GB/s HBM write stream is the real roofline,
8.4 MB -> ~23.3 us, which CoreSim's cross-engine DMA overlap undercounts):
- Output DMAs alternate between the two physical HWDGE rings (SP/nc.sync and
  ACT/nc.scalar); the last chunk goes out via Pool/SWDGE, by which point the
  engines are otherwise idle. SWDGE for the whole stream (the baseline)
  serializes descriptor generation on Pool and its descriptor rings share
  SBUF ports with DVE's 2-port perf mode; HWDGE frees the sequencer before
  the transfer, and spreading rings lets DMA k+1's issue/wait overlap DMA
  k's transfer.
- The bias DMA's end-to-end latency (~2.4 us: hwdge + dge delay + sem prop)
  is the unavoidable head of the critical path (per-core data cannot be an
  immediate: one SPMD program serves all 8 cores); the J iotas run on Pool
  in parallel with it.
- Output is cut into chunks, small first (256, 1536, 2304 free elems, then
  three full 4096 tiles): the first DMA issues ~2.8 us in. Compute is spread
  over THREE engines: DVE does all c=0 planes plus chunk1's c=1; Pool (idle
  after the iotas) does c=1 of chunks 2-4; ACT does c=1 of chunks 0 and 5 as
  Identity activations (out = Identity(J*scale + bias), bias as a
  per-partition AP) in its idle window before its first DMA — chunk5's c1
  is emitted ahead of that DMA in OP_SCHEDULE, which re-gates Pool's final
  DMA onto the DVE c0 chain instead of Pool's own c1 chain. A no-dep ACT
  warmup op at kernel start overlaps the ~1.3 us activation-table load with
  the bias DMA (omitting it costs ~1.1 us).
- Every chunk gets its own SBUF buffer (no slot reuse -> no WAR waits); each
  output DMA's data dependency fits walrus's single-wait HWDGE slot (multi-
  plane-engine chunks need a second wait, split into a Drain carrier by
  _legalize_waits). 7 total DMAs <= 8 DMAHW lanes, so no lane reuse.

Sync-wait slot limits in walrus codegen (HWDGE DMA: 1, DVE/ACT: 2):
_legalize_waits splits any over-limit instruction (the Tile kernel-tail
drain) into single-wait Drain carriers.
"""

import numpy as np

NSTEPS = 2
NSYMS = 2048
NCORES = 8
A_PER_CORE = NSYMS // NCORES          # 256
BLKS = A_PER_CORE // 128              # 2 partition blocks per core
F = NSYMS * 2                         # 4096 free elements per a-row (b, c interleaved)

# Per-tile f-split sizes, applied tile-by-tile in (t, blk) order: small chunks
# first so the DMA stream starts early; all bounds even (c-pairs).
CHUNK_SIZES = [256, 1536, 2304, 4096, 4096, 4096]

# J iota pieces (b ranges) sized so each chunk's J slice is ready before the
# bias DMA lands.
J_SPLITS = [(0, 128), (128, 640), (640, 2048)]

# Engines that issue the output DMAs; both SP ("sync") and ACT ("scalar")
# have HWDGE rings, and alternating engines lets DMA k+1's issue/wait overlap
# DMA k's transfer. If the list is shorter than the chunk list it is applied
# round-robin; a list as long as CHUNK_SIZES assigns engines per chunk.
OUT_DMA_ENGINES = ["sync", "scalar", "sync", "scalar", "sync", "gpsimd"]

# Engine for the bias input DMA.
BIAS_DMA_ENGINE = "sync"

# Per-chunk engine for the c=1 plane compute ("vector" = DVE, "gpsimd" = Pool,
# "scalar" = ACT via Identity activation). Pool is idle after the iotas, so
# giving it the big chunks' c=1 planes shortens the serial compute chain;
# c=0 always stays on DVE. None -> all DVE.
C1_ENGINES = ["scalar", "vector", "gpsimd", "gpsimd", "gpsimd", "scalar"]

# Emit a tiny ACT Identity op with no data deps at kernel start so the ACT
# activation-table load (~1.3 us) overlaps the bias DMA instead of delaying
# the first real ACT compute. Only useful when C1_ENGINES contains "scalar".
ACT_WARMUP = True

# Optional explicit emission order: list of ("cmp", chunk, c) / ("dma", chunk)
# entries covering every plane and DMA exactly once. None -> per-chunk order
# (c0, c1, dma per chunk). Engine FIFO order follows emission order; the one
# deviation from per-chunk order is chunk5's c1 emitted before ACT's first
# DMA, so it runs in ACT's early idle window and unblocks Pool's final DMA
# (gated by DVE's c0 chain instead of Pool's own c1 chain).
OP_SCHEDULE = [
    ("cmp", 0, 0), ("cmp", 0, 1), ("dma", 0),
    ("cmp", 1, 0), ("cmp", 1, 1), ("cmp", 5, 1), ("dma", 1),
    ("cmp", 2, 0), ("cmp", 2, 1), ("dma", 2),
    ("cmp", 3, 0), ("cmp", 3, 1), ("dma", 3),
    ("cmp", 4, 0), ("cmp", 4, 1), ("dma", 4),
    ("cmp", 5, 0), ("dma", 5),
]


def _chunks():
    tiles = [(t, blk) for t in range(NSTEPS) for blk in range(BLKS)]
    out, ti, f = [], 0, 0
    for sz in CHUNK_SIZES:
        t, blk = tiles[ti]
        out.append((t, blk, f, f + sz))
        f += sz
        if f == F:
            ti, f = ti + 1, 0
    assert ti == len(tiles) and f == 0, "CHUNK_SIZES must tile 4 x F exactly"
    return out

_CACHE = {}


def _build_bass(scales, legalize=True):
    """legalize=False builds the pre-legalization twin of the program: walrus
    cannot compile it (multi-wait instructions), but CoreSim times it
    faithfully — the wait-split Drain carriers confuse CoreSim's queue model
    into starting some ops before their semaphore waits are satisfied, so the
    legalized build's sim time is optimistic garbage. Use the twin for
    timing, the legalized build for hardware."""
    import concourse.bass as bass
    import concourse.mybir as mybir
    from concourse.tile import TileContext

    f32 = mybir.dt.float32
    nc = bass.Bass(trn_type="TRN2")

    bias_in = nc.dram_tensor("bias_in", [128, NSTEPS * BLKS * 2], f32, kind="ExternalInput")
    out = nc.dram_tensor("out", [NSTEPS, BLKS, 128, F], f32, kind="ExternalOutput")

    chunks = _chunks()
    with TileContext(nc) as tc:
        with (
            tc.tile_pool(name="const", bufs=1) as const,
            tc.tile_pool(name="outp", bufs=len(chunks)) as outp,
        ):
            bias_sb = const.tile([128, NSTEPS * BLKS * 2], f32)
            getattr(nc, BIAS_DMA_ENGINE).dma_start(bias_sb[:], bias_in[:])

            if ACT_WARMUP:
                warm = const.tile([128, 2], f32, name="warm")
                nc.vector.memset(warm[:], 0.0)
                nc.scalar.activation(
                    warm[:], warm[:],
                    func=mybir.ActivationFunctionType.Identity,
                    bias=0.0, scale=1.0,
                )

            J = const.tile([128, NSYMS], f32)
            for b0, b1 in J_SPLITS:
                nc.gpsimd.iota(
                    J[:, b0:b1], pattern=[[1, b1 - b0]], base=b0,
                    channel_multiplier=0, allow_small_or_imprecise_dtypes=True,
                )

            tiles = {}
            for k, (t, blk, f0, f1) in enumerate(chunks):
                tiles[k] = outp.tile([128, f1 - f0], f32, tag="ot", name=f"ot{k}")

            if OP_SCHEDULE is None:
                sched = []
                for k in range(len(chunks)):
                    sched += [("cmp", k, 0), ("cmp", k, 1), ("dma", k)]
            else:
                sched = OP_SCHEDULE

            for op in sched:
                k = op[1]
                t, blk, f0, f1 = chunks[k]
                ot = tiles[k]
                if op[0] == "cmp":
                    c = op[2]
                    idx = (t * BLKS + blk) * 2 + c
                    otv = ot[:].rearrange("p (b c) -> p b c", c=2)
                    c1_eng = C1_ENGINES[k] if C1_ENGINES else "vector"
                    eng_name = "vector" if c == 0 else c1_eng
                    if eng_name == "scalar":
                        # ACT has no tensor_scalar; Identity(in*scale + bias)
                        # computes the same J*s + bias with a per-partition
                        # bias AP.
                        nc.scalar.activation(
                            otv[:, :, c],
                            J[:, f0 // 2 : f1 // 2],
                            func=mybir.ActivationFunctionType.Identity,
                            bias=bias_sb[:, idx : idx + 1],
                            scale=scales[c],
                        )
                    else:
                        getattr(nc, eng_name).tensor_scalar(
                            otv[:, :, c],
                            J[:, f0 // 2 : f1 // 2],
                            scales[c],
                            bias_sb[:, idx : idx + 1],
                            mybir.AluOpType.mult,
                            mybir.AluOpType.add,
                        )
                else:
                    eng = OUT_DMA_ENGINES[k % len(OUT_DMA_ENGINES)]
                    getattr(nc, eng).dma_start(out[t, blk, :, f0:f1], ot[:])

    if legalize:
        _legalize_waits(nc, mybir)
    return nc


def _legalize_waits(nc, mybir):
    """This walrus build fits very few semaphore waits per instruction (one
    for most engine structs). Tile's auto-generated kernel-tail drain waits
    on every DMA lane + engine sem at once; split any multi-wait instruction
    into a chain of single-wait Drain carriers on the same engine."""
    for func in nc.m.functions:
        for block in func.blocks:
            insts = list(block.instructions)
            new_insts = []
            changed = False
            for inst in insts:
                si = inst.sync_info
                waits = list(si.on_wait) if si is not None and si.on_wait else []
                if len(waits) > 1:
                    for w in waits[:-1]:
                        d = mybir.InstDrain(
                            name=f"{inst.name}-waitsplit-{len(new_insts)}",
                            ins=[],
                            outs=[],
                            bass_is_fusable=False,
                        )
                        d.engine = inst.engine
                        d.sync_info = mybir.SyncInfo(on_wait=[w], on_update=[])
                        new_insts.append(d)
                    inst.sync_info = mybir.SyncInfo(
                        on_wait=[waits[-1]], on_update=list(si.on_update or [])
                    )
                    changed = True
                new_insts.append(inst)
            if changed:
                block.instructions = new_insts


def _host_consts(gb, w_hat1, m_hat1, w_hat2, m_hat2, w_hat3, m_hat3):
    def nacw(w, m):
        w = np.asarray(w, np.float64)
        m = np.asarray(m, np.float64)
        return np.tanh(w) * (1.0 / (1.0 + np.exp(-m)))

    weff = nacw(w_hat3, m_hat3) @ nacw(w_hat2, m_hat2) @ nacw(w_hat1, m_hat1)  # [2,3]
    gb = np.asarray(gb, np.float64)

    scales = [float(np.float32(weff[c, 2] / NSYMS)) for c in range(2)]

    # bias[core][p, (t,blk,c)] = gb[c] + (t/2)Weff[c,0] + (a/2048)Weff[c,1]
    biases = []
    for core in range(NCORES):
        bias = np.empty((128, NSTEPS, BLKS, 2), np.float64)
        for t in range(NSTEPS):
            for blk in range(BLKS):
                a = (core * A_PER_CORE + blk * 128 + np.arange(128)) / NSYMS
                for c in range(2):
                    bias[:, t, blk, c] = (
                        gb[c] + (t / NSTEPS) * weff[c, 0] + a * weff[c, 1]
                    )
        biases.append(np.ascontiguousarray(bias.reshape(128, -1), np.float32))
    return scales, biases


def kernel(market, gb, w_hat1, m_hat1, w_hat2, m_hat2, w_hat3, m_hat3):
    from concourse.bass_utils import run_bass_kernel_spmd

    scales, biases = _host_consts(gb, w_hat1, m_hat1, w_hat2, m_hat2, w_hat3, m_hat3)
    # the tensor_scalar immediates (scales) are baked into the traced program,
    # so the compiled module is keyed on them
    key = ("nc", tuple(scales))
    if key not in _CACHE:
        _CACHE[key] = _build_bass(scales)
    nc = _CACHE[key]
    _CACHE["last_nc"] = nc

    in_maps = [{"bias_in": biases[core]} for core in range(NCORES)]
    res = run_bass_kernel_spmd(nc, in_maps, core_ids=list(range(NCORES)))
    parts = [r["out"].reshape(NSTEPS, A_PER_CORE, NSYMS, 2) for r in res.results]
    return np.concatenate(parts, axis=1)


# revision 29
# speedup vs baseline: 1.0468x; 1.0468x over previous
"""Trainium2 Bass kernel for nn_ReallocationMapEncoder.

The reference network is three NAC layers (y = x @ (tanh(W_hat)*sigmoid(M_hat)).T)
applied to a [nsteps, nsyms, nsyms, 3] grid of normalized (t, a, b) indices,
plus a gb broadcast on the trailing axis. NAC is linear in x, so the whole
network collapses to one effective matrix Weff = W3 @ W2 @ W1 of shape [2, 3]:

    y[t, a, b, c] = gb[c] + (t/2)*Weff[c,0] + (a/2048)*Weff[c,1] + (b/2048)*Weff[c,2]

The output [2, 2048, 2048, 2] f32 (67 MB) is a separable affine ramp; the kernel
is purely output-write-bandwidth bound (memory regime).

Device strategy (8 cores, data-parallel on the `a` axis, 256 rows each, so each
core writes 8.4 MB): a DVE iota builds J[p, j] = j; every output chunk
[128 a-rows, fsz free elems at c-stride] is a fused DVE tensor_scalar

    out[p, b, c] = J[p, b] * (Weff[c,2]/nsyms) + bias[p, (t,blk,c)]

where bias (a tiny [128, 8] per-core input) folds the gb/t/a terms.

Perf structure (16.4 us on the CoreSim cost model vs 32.6 us for the
all-SWDGE baseline; the ~358 GB/s HBM write stream is the real roofline,
8.4 MB -> ~23.3 us, which CoreSim's cross-engine DMA overlap undercounts):
- Output DMAs alternate between the two physical HWDGE rings (SP/nc.sync and
  ACT/nc.scalar); the last chunk goes out via Pool/SWDGE, by which point the
  engines are otherwise idle. SWDGE for the whole stream (the baseline)
  serializes descriptor generation on Pool and its descriptor rings share
  SBUF ports with DVE's 2-port perf mode; HWDGE frees the sequencer before
  the transfer, and spreading rings lets DMA k+1's issue/wait overlap DMA
  k's transfer.
- The bias DMA's end-to-end latency (~2.4 us: hwdge + dge delay + sem prop)
  is the unavoidable head of the critical path (per-core data cannot be an
  immediate: one SPMD program serves all 8 cores); the J iotas run on Pool
  in parallel with it.
- Output is cut into chunks, small first (256, 1536, 2304 free elems, then
  three full 4096 tiles): the first DMA issues ~2.8 us in. Compute is spread
  over THREE engines: DVE does all c=0 planes plus chunk1's c=1; Pool (idle
  after the iotas) does c=1 of chunks 2-4; ACT does c=1 of chunks 0 and 5 as
  Identity activations (out = Identity(J*scale + bias), bias as a
  per-partition AP) in its idle window before its first DMA — chunk5's c1
  is emitted ahead of that DMA in OP_SCHEDULE, which re-gates Pool's final
  DMA onto the DVE c0 chain instead of Pool's own c1 chain. A no-dep ACT
  warmup op at kernel start overlaps the ~1.3 us activation-table load with
  the bias DMA (omitting it costs ~1.1 us).
- Every chunk gets its own SBUF buffer (no slot reuse -> no WAR waits); each
  output DMA's data dependency fits walrus's single-wait HWDGE slot (multi-
  plane-engine chunks need a second wait, split into a Drain carrier by
  _legalize_waits). 7 total DMAs <= 8 DMAHW lanes, so no lane reuse.

Sync-wait slot limits in walrus codegen (HWDGE DMA: 1, DVE/ACT: 2):
_legalize_waits splits any over-limit instruction (the Tile kernel-tail
drain) into single-wait Drain carriers.
"""

import numpy as np

NSTEPS = 2
NSYMS = 2048
NCORES = 8
A_PER_CORE = NSYMS // NCORES          # 256
BLKS = A_PER_CORE // 128              # 2 partition blocks per core
F = NSYMS * 2                         # 4096 free elements per a-row (b, c interleaved)

# Per-tile f-split sizes, applied tile-by-tile in (t, blk) order: small chunks
# first so the DMA stream starts early; all bounds even (c-pairs).
CHUNK_SIZES = [256, 1536, 2304, 4096, 4096, 4096]

# J iota pieces (b ranges) sized so each chunk's J slice is ready before the
# bias DMA lands.
J_SPLITS = [(0, 128), (128, 640), (640, 2048)]

# Engines that issue the output DMAs; both SP ("sync") and ACT ("scalar")
# have HWDGE rings, and alternating engines lets DMA k+1's issue/wait overlap
# DMA k's transfer. If the list is shorter than the chunk list it is applied
# round-robin; a list as long as CHUNK_SIZES assigns engines per chunk.
OUT_DMA_ENGINES = ["sync", "scalar", "sync", "scalar", "sync", "gpsimd"]

# Engine for the bias input DMA.
BIAS_DMA_ENGINE = "sync"

# Per-chunk engine for the c=1 plane compute ("vector" = DVE, "gpsimd" = Pool,
# "scalar" = ACT via Identity activation). Pool is idle after the iotas, so
# giving it the big chunks' c=1 planes shortens the serial compute chain;
# c=0 always stays on DVE. None -> all DVE.
C1_ENGINES = ["scalar", "vector", "gpsimd", "gpsimd", "gpsimd", "scalar"]

# Emit a tiny ACT Identity op with no data deps at kernel start so the ACT
# activation-table load (~1.3 us) overlaps the bias DMA instead of delaying
# the first real ACT compute. Only useful when C1_ENGINES contains "scalar".
ACT_WARMUP = True

# Optional explicit emission order: list of ("cmp", chunk, c) / ("dma", chunk)
# entries covering every plane and DMA exactly once. None -> per-chunk order
# (c0, c1, dma per chunk). Engine FIFO order follows emission order; the one
# deviation from per-chunk order is chunk5's c1 emitted before ACT's first
# DMA, so it runs in ACT's early idle window and unblocks Pool's final DMA
# (gated by DVE's c0 chain instead of Pool's own c1 chain).
# Found by randomized search over valid emission orders (60k samples against
# the cost model): 15,677 ns vs 16,411 for the best hand-crafted schedule.
OP_SCHEDULE = [
    ("cmp", 0, 0), ("cmp", 2, 1), ("cmp", 0, 1), ("cmp", 2, 0), ("cmp", 3, 1), ("dma", 2),
    ("cmp", 5, 0), ("cmp", 4, 0), ("cmp", 5, 1), ("cmp", 1, 1), ("cmp", 3, 0), ("cmp", 1, 0),
    ("dma", 3), ("dma", 1), ("dma", 0), ("cmp", 4, 1), ("dma", 5), ("dma", 4),
]


def _chunks():
    tiles = [(t, blk) for t in range(NSTEPS) for blk in range(BLKS)]
    out, ti, f = [], 0, 0
    for sz in CHUNK_SIZES:
        t, blk = tiles[ti]
        out.append((t, blk, f, f + sz))
        f += sz
        if f == F:
            ti, f = ti + 1, 0
    assert ti == len(tiles) and f == 0, "CHUNK_SIZES must tile 4 x F exactly"
    return out

_CACHE = {}


def _build_bass(scales, legalize=True):
    """legalize=False builds the pre-legalization twin of the program: walrus
    cannot compile it (multi-wait instructions), but CoreSim times it
    faithfully — the wait-split Drain carriers confuse CoreSim's queue model
    into starting some ops before their semaphore waits are satisfied, so the
    legalized build's sim time is optimistic garbage. Use the twin for
    timing, the legalized build for hardware."""
    import concourse.bass as bass
    import concourse.mybir as mybir
    from concourse.tile import TileContext

    f32 = mybir.dt.float32
    nc = bass.Bass(trn_type="TRN2")

    bias_in = nc.dram_tensor("bias_in", [128, NSTEPS * BLKS * 2], f32, kind="ExternalInput")
    out = nc.dram_tensor("out", [NSTEPS, BLKS, 128, F], f32, kind="ExternalOutput")

    chunks = _chunks()
    with TileContext(nc) as tc:
        with (
            tc.tile_pool(name="const", bufs=1) as const,
            tc.tile_pool(name="outp", bufs=len(chunks)) as outp,
        ):
            bias_sb = const.tile([128, NSTEPS * BLKS * 2], f32)
            getattr(nc, BIAS_DMA_ENGINE).dma_start(bias_sb[:], bias_in[:])

            if ACT_WARMUP:
                warm = const.tile([128, 2], f32, name="warm")
                nc.vector.memset(warm[:], 0.0)
                nc.scalar.activation(
                    warm[:], warm[:],
                    func=mybir.ActivationFunctionType.Identity,
                    bias=0.0, scale=1.0,
                )

            J = const.tile([128, NSYMS], f32)
            for b0, b1 in J_SPLITS:
                nc.gpsimd.iota(
                    J[:, b0:b1], pattern=[[1, b1 - b0]], base=b0,
                    channel_multiplier=0, allow_small_or_imprecise_dtypes=True,
                )

            tiles = {}
            for k, (t, blk, f0, f1) in enumerate(chunks):
                tiles[k] = outp.tile([128, f1 - f0], f32, tag="ot", name=f"ot{k}")

            if OP_SCHEDULE is None:
                sched = []
                for k in range(len(chunks)):
                    sched += [("cmp", k, 0), ("cmp", k, 1), ("dma", k)]
            else:
                sched = OP_SCHEDULE

            for op in sched:
                k = op[1]
                t, blk, f0, f1 = chunks[k]
                ot = tiles[k]
                if op[0] == "cmp":
                    c = op[2]
                    idx = (t * BLKS + blk) * 2 + c
                    otv = ot[:].rearrange("p (b c) -> p b c", c=2)
                    c1_eng = C1_ENGINES[k] if C1_ENGINES else "vector"
                    eng_name = "vector" if c == 0 else c1_eng
                    if eng_name == "scalar":
                        # ACT has no tensor_scalar; Identity(in*scale + bias)
                        # computes the same J*s + bias with a per-partition
                        # bias AP.
                        nc.scalar.activation(
                            otv[:, :, c],
                            J[:, f0 // 2 : f1 // 2],
                            func=mybir.ActivationFunctionType.Identity,
                            bias=bias_sb[:, idx : idx + 1],
                            scale=scales[c],
                        )
                    else:
                        getattr(nc, eng_name).tensor_scalar(
                            otv[:, :, c],
                            J[:, f0 // 2 : f1 // 2],
                            scales[c],
                            bias_sb[:, idx : idx + 1],
                            mybir.AluOpType.mult,
                            mybir.AluOpType.add,
                        )
                else:
                    eng = OUT_DMA_ENGINES[k % len(OUT_DMA_ENGINES)]
                    getattr(nc, eng).dma_start(out[t, blk, :, f0:f1], ot[:])

    if legalize:
        _legalize_waits(nc, mybir)
    return nc


def _legalize_waits(nc, mybir):
    """This walrus build fits very few semaphore waits per instruction (one
    for most engine structs). Tile's auto-generated kernel-tail drain waits
    on every DMA lane + engine sem at once; split any multi-wait instruction
    into a chain of single-wait Drain carriers on the same engine."""
    for func in nc.m.functions:
        for block in func.blocks:
            insts = list(block.instructions)
            new_insts = []
            changed = False
            for inst in insts:
                si = inst.sync_info
                waits = list(si.on_wait) if si is not None and si.on_wait else []
                if len(waits) > 1:
                    for w in waits[:-1]:
                        d = mybir.InstDrain(
                            name=f"{inst.name}-waitsplit-{len(new_insts)}",
                            ins=[],
                            outs=[],
                            bass_is_fusable=False,
                        )
                        d.engine = inst.engine
                        d.sync_info = mybir.SyncInfo(on_wait=[w], on_update=[])
                        new_insts.append(d)
                    inst.sync_info = mybir.SyncInfo(
                        on_wait=[waits[-1]], on_update=list(si.on_update or [])
                    )
                    changed = True
                new_insts.append(inst)
            if changed:
                block.instructions = new_insts


def _host_consts(gb, w_hat1, m_hat1, w_hat2, m_hat2, w_hat3, m_hat3):
    def nacw(w, m):
        w = np.asarray(w, np.float64)
        m = np.asarray(m, np.float64)
        return np.tanh(w) * (1.0 / (1.0 + np.exp(-m)))

    weff = nacw(w_hat3, m_hat3) @ nacw(w_hat2, m_hat2) @ nacw(w_hat1, m_hat1)  # [2,3]
    gb = np.asarray(gb, np.float64)

    scales = [float(np.float32(weff[c, 2] / NSYMS)) for c in range(2)]

    # bias[core][p, (t,blk,c)] = gb[c] + (t/2)Weff[c,0] + (a/2048)Weff[c,1]
    biases = []
    for core in range(NCORES):
        bias = np.empty((128, NSTEPS, BLKS, 2), np.float64)
        for t in range(NSTEPS):
            for blk in range(BLKS):
                a = (core * A_PER_CORE + blk * 128 + np.arange(128)) / NSYMS
                for c in range(2):
                    bias[:, t, blk, c] = (
                        gb[c] + (t / NSTEPS) * weff[c, 0] + a * weff[c, 1]
                    )
        biases.append(np.ascontiguousarray(bias.reshape(128, -1), np.float32))
    return scales, biases


def kernel(market, gb, w_hat1, m_hat1, w_hat2, m_hat2, w_hat3, m_hat3):
    from concourse.bass_utils import run_bass_kernel_spmd

    scales, biases = _host_consts(gb, w_hat1, m_hat1, w_hat2, m_hat2, w_hat3, m_hat3)
    # the tensor_scalar immediates (scales) are baked into the traced program,
    # so the compiled module is keyed on them
    key = ("nc", tuple(scales))
    if key not in _CACHE:
        _CACHE[key] = _build_bass(scales)
    nc = _CACHE[key]
    _CACHE["last_nc"] = nc

    in_maps = [{"bias_in": biases[core]} for core in range(NCORES)]
    res = run_bass_kernel_spmd(nc, in_maps, core_ids=list(range(NCORES)))
    parts = [r["out"].reshape(NSTEPS, A_PER_CORE, NSYMS, 2) for r in res.results]
    return np.concatenate(parts, axis=1)
